# revision 53
# baseline (speedup 1.0000x reference)
"""Trainium2 Bass kernel for multi-head attention (B=4, N=M=1024, C=768, H=12).

Sharding: 8 cores = 4 batches x 2 head-groups (6 heads each).
Each core computes, for its (batch b, head-group g):
  qhT = (Wq[:, g])^T-projection in [head_dim, tokens] layout     [384, 1024]
  khT = same for k                                               [384, 1024]
  vh  = v @ Wv[:, g] in [tokens, head_dim] layout                [1024, 384]
  per head h (pairs packed via tile_position):
    S^T = kh_h @ qh_h^T                (k-tokens on partitions)
    E   = exp(S^T * 0.125)            (ScalarE, scale fused)
    rowsum_h[q] = ones^T @ E          (PE column-sum, PSUM-accumulated)
    P   = E * mask^T                  (VectorE)
    OT_h += vh_h^T-chunks @ P         (PSUM-accumulated over k-tiles)
  OT'_h = OT_h * (1/rowsum_h)         (recip broadcast via gpsimd)
  Ypart = OT' @ Wp[g-rows, :]         (partial output projection)
Host: Y[b] = Ypart[2b] + Ypart[2b+1] + bp.

All matmuls run in float32r (full-rate fp32, ~1.5e-4 rel err per matmul).
"""

import numpy as np
from contextlib import ExitStack

import concourse.bass as bass
import concourse.tile as tile
from concourse import bacc, mybir
from concourse import bass_utils

F32 = mybir.dt.float32
F32R = mybir.dt.float32r
EXP = mybir.ActivationFunctionType.Exp

C = 768          # model dim
T = 1024         # tokens (N == M)
HG = 384         # head dims per core (6 heads x 64)
D = 64           # head dim
SCALE = 0.125    # 1/sqrt(64)

_CACHE = {}


def _emit(tc, nc, a):
    """Emit the per-core program. `a` holds the DRAM APs."""
    ctx = ExitStack()
    with ctx:
        # ---- SBUF pools ----
        wpool = ctx.enter_context(tc.tile_pool(name="w", bufs=1))
        mpool = ctx.enter_context(tc.tile_pool(name="mask", bufs=1))
        hpool = ctx.enter_context(tc.tile_pool(name="heads", bufs=1))
        cpool = ctx.enter_context(tc.tile_pool(name="consts", bufs=1))

        wq = [wpool.tile([128, HG], F32R, name=f"wq{c}", tag=f"wq{c}") for c in range(6)]
        wk = [wpool.tile([128, HG], F32R, name=f"wk{c}", tag=f"wk{c}") for c in range(6)]
        wv = [wpool.tile([128, HG], F32R, name=f"wv{c}", tag=f"wv{c}") for c in range(6)]
        wp = [wpool.tile([128, C], F32R, name=f"wp{r}", tag=f"wp{r}") for r in range(3)]
        maskT = [mpool.tile([128, T], F32, name=f"m{t}", tag=f"m{t}") for t in range(8)]
        qhT = [hpool.tile([128, T], F32R, name=f"qh{m}", tag=f"qh{m}") for m in range(3)]
        khT = [hpool.tile([128, T], F32R, name=f"kh{m}", tag=f"kh{m}") for m in range(3)]
        vh = [hpool.tile([128, HG], F32R, name=f"vh{t}", tag=f"vh{t}") for t in range(8)]
        ot_sb = [hpool.tile([128, T], F32R, name=f"ot{m}", tag=f"ot{m}") for m in range(3)]
        ones = cpool.tile([128, 1], F32R, name="ones", tag="ones")

        # projection-phase pools (closed before attention pools open)
        pctx = ExitStack()
        ipool = pctx.enter_context(tc.tile_pool(name="in", bufs=1))
        qT = [ipool.tile([128, T], F32R, name=f"qT{c}", tag=f"qT{c}") for c in range(6)]
        kT = [ipool.tile([128, T], F32R, name=f"kT{c}", tag=f"kT{c}") for c in range(6)]
        vpool = pctx.enter_context(tc.tile_pool(name="vin", bufs=1))
        vT = [vpool.tile([128, T], F32R, name=f"vT{c}", tag=f"vT{c}") for c in range(6)]
        prj = pctx.enter_context(tc.tile_pool(name="prj", bufs=4, space="PSUM"))
        vps = pctx.enter_context(tc.tile_pool(name="vps", bufs=4, space="PSUM"))

        # DMA issue order == need order
        nc.sync.dma_start(ones[:], a["ones"][:])
        for c in range(6):
            nc.sync.dma_start(wq[c][:], a["wq"][c * 128:(c + 1) * 128, :])
            nc.sync.dma_start(qT[c][:], a["qT"][c * 128:(c + 1) * 128, :])
        for c in range(6):
            nc.sync.dma_start(wk[c][:], a["wk"][c * 128:(c + 1) * 128, :])
            nc.sync.dma_start(kT[c][:], a["kT"][c * 128:(c + 1) * 128, :])
        for c in range(6):
            nc.sync.dma_start(wv[c][:], a["wv"][c * 128:(c + 1) * 128, :])
            nc.sync.dma_start(vT[c][:], a["vT"][c * 128:(c + 1) * 128, :])
        for t in range(8):
            nc.sync.dma_start(maskT[t][:], a["maskT"][t * 128:(t + 1) * 128, :])
        for r in range(3):
            nc.sync.dma_start(wp[r][:], a["wp"][r * 128:(r + 1) * 128, :])

        # ---- all projections upfront ----
        for m in range(3):
            for (w_t, in_t, dst) in ((wq, qT, qhT[m]), (wk, kT, khT[m])):
                for half in range(2):
                    hs = slice(half * 512, (half + 1) * 512)
                    ps = prj.tile([128, 512], F32, name="proj", tag="proj")
                    for c in range(6):
                        nc.tensor.matmul(
                            ps[:], w_t[c][:, m * 128:(m + 1) * 128],
                            in_t[c][:, hs], start=(c == 0), stop=(c == 5))
                    nc.scalar.copy(dst[:, hs], ps[:])
        for t in range(8):
            ps = vps.tile([128, HG], F32, name="vproj", tag="vproj")
            for c in range(6):
                nc.tensor.matmul(
                    ps[:], vT[c][:, t * 128:(t + 1) * 128], wv[c][:],
                    start=(c == 0), stop=(c == 5))
            nc.scalar.copy(vh[t][:], ps[:])
        pctx.close()

        # ---- attention: merged head-pair S/E tiles, one exp per k ----
        actx = ExitStack()
        e_pool = actx.enter_context(tc.tile_pool(name="e", bufs=8))
        p_pool = actx.enter_context(tc.tile_pool(name="p", bufs=6))
        r_pool = actx.enter_context(tc.tile_pool(name="recip", bufs=4))
        b_pool = actx.enter_context(tc.tile_pool(name="bc_sb", bufs=4))
        s_ps = actx.enter_context(tc.tile_pool(name="s_ps", bufs=2, space="PSUM"))
        rs_ps = actx.enter_context(tc.tile_pool(name="rs_ps", bufs=2, space="PSUM"))
        ot_ps = actx.enter_context(tc.tile_pool(name="ot_ps", bufs=2, space="PSUM"))

        def emit_attn(hp, qh):
            qs = slice(qh * 512, (qh + 1) * 512)
            rs = [rs_ps.tile([1, 512], F32, name="rs", tag="rs") for _ in range(2)]
            ot01 = [ot_ps.tile([64, 512], F32, name="ot", tag="ot") for _ in range(2)]
            for k in range(8):
                ks = slice(k * 128, (k + 1) * 128)
                s = s_ps.tile([128, 1024], F32, name="s", tag="s")
                for h in range(2):
                    hs = slice(h * 64, (h + 1) * 64)
                    nc.tensor.matmul(
                        s[:, h * 512:(h + 1) * 512], khT[hp][hs, ks],
                        qhT[hp][hs, qs],
                        start=True, stop=True, tile_position=(h * 64, 0))
                e = e_pool.tile([128, 1024], F32R, name="e", tag="e")
                nc.scalar.activation(e[:], s[:], EXP, scale=SCALE)
                for h in range(2):
                    eh = e[:, h * 512:(h + 1) * 512]
                    nc.tensor.matmul(rs[h][:], ones[:], eh,
                                     start=(k == 0), stop=(k == 7))
                p = p_pool.tile([128, 1024], F32R, name="p", tag="p")
                msl = maskT[k][:, qs]
                mrep = bass.AP(msl.tensor, msl.offset,
                               [list(msl.ap[0]), [0, 2], [1, 512]])
                e3 = bass.AP(e.tensor, e.offset,
                             [list(e[:].ap[0]), [512, 2], [1, 512]])
                p3 = bass.AP(p.tensor, p.offset,
                             [list(p[:].ap[0]), [512, 2], [1, 512]])
                nc.vector.tensor_mul(p3, e3, mrep)
                for h in range(2):
                    nc.tensor.matmul(
                        ot01[h][:],
                        vh[k][:, hp * 128 + h * 64: hp * 128 + (h + 1) * 64],
                        p[:, h * 512:(h + 1) * 512],
                        start=(k == 0), stop=(k == 7))
            for h in range(2):
                recip = r_pool.tile([1, 512], F32, name="recip", tag="recip")
                nc.vector.reciprocal(recip[:], rs[h][:])
                bc_s = b_pool.tile([64, 512], F32, name="bc_s", tag="bc_s")
                nc.gpsimd.partition_broadcast(bc_s[:], recip[:])
                nc.vector.tensor_mul(
                    ot_sb[hp][h * 64:(h + 1) * 64, qs], ot01[h][:], bc_s[:])

        for qh in range(2):
            for hp in range(3):
                emit_attn(hp, qh)
        actx.close()

        # ---- output projection ----
        with ExitStack() as yctx:
            y_ps = yctx.enter_context(tc.tile_pool(name="y_ps", bufs=2, space="PSUM"))
            y_pool = yctx.enter_context(tc.tile_pool(name="y_sb", bufs=2))
            for t in range(8):
                ts = slice(t * 128, (t + 1) * 128)
                y = y_ps.tile([128, C], F32, name="y", tag="y")
                for r in range(3):
                    nc.tensor.matmul(y[:, 0:512], ot_sb[r][:, ts], wp[r][:, 0:512],
                                     start=(r == 0), stop=(r == 2))
                    nc.tensor.matmul(y[:, 512:768], ot_sb[r][:, ts], wp[r][:, 512:768],
                                     start=(r == 0), stop=(r == 2))
                y_sb = y_pool.tile([128, C], F32, name="y_sb", tag="y_sb")
                nc.scalar.copy(y_sb[:], y[:])
                nc.sync.dma_start(a["y"][ts, :], y_sb[:])


def _build():
    nc = bacc.Bacc("TRN2", target_bir_lowering=False, debug=False, num_devices=8)
    a = {
        "qT": nc.dram_tensor("qT", [C, T], F32R, kind="ExternalInput").ap(),
        "kT": nc.dram_tensor("kT", [C, T], F32R, kind="ExternalInput").ap(),
        "vT": nc.dram_tensor("vT", [C, T], F32R, kind="ExternalInput").ap(),
        "maskT": nc.dram_tensor("maskT", [T, T], F32, kind="ExternalInput").ap(),
        "wq": nc.dram_tensor("wq", [C, HG], F32R, kind="ExternalInput").ap(),
        "wk": nc.dram_tensor("wk", [C, HG], F32R, kind="ExternalInput").ap(),
        "wv": nc.dram_tensor("wv", [C, HG], F32R, kind="ExternalInput").ap(),
        "wp": nc.dram_tensor("wp", [HG, C], F32R, kind="ExternalInput").ap(),
        "ones": nc.dram_tensor("ones", [128, 1], F32R, kind="ExternalInput").ap(),
        "y": nc.dram_tensor("y", [T, C], F32, kind="ExternalOutput").ap(),
    }
    with tile.TileContext(nc) as tc:
        _emit(tc, nc, a)
    nc.compile()
    return nc


def _in_maps(q, k, v, mask, Wq, Wk, Wv, Wp):
    # per-batch tensors are shared (read-only) by the two cores of a batch
    qT = [np.ascontiguousarray(q[b].T, dtype=np.float32) for b in range(4)]
    kT = [np.ascontiguousarray(k[b].T, dtype=np.float32) for b in range(4)]
    vT = [np.ascontiguousarray(v[b].T, dtype=np.float32) for b in range(4)]
    mT = [np.ascontiguousarray(mask[b].T, dtype=np.float32) for b in range(4)]
    wq = [np.ascontiguousarray(Wq[:, HG * g:HG * (g + 1)], dtype=np.float32)
          for g in range(2)]
    wk = [np.ascontiguousarray(Wk[:, HG * g:HG * (g + 1)], dtype=np.float32)
          for g in range(2)]
    wv = [np.ascontiguousarray(Wv[:, HG * g:HG * (g + 1)], dtype=np.float32)
          for g in range(2)]
    wp = [np.ascontiguousarray(Wp[HG * g:HG * (g + 1), :], dtype=np.float32)
          for g in range(2)]
    ones = np.ones((128, 1), np.float32)
    return [{
        "qT": qT[core // 2], "kT": kT[core // 2], "vT": vT[core // 2],
        "maskT": mT[core // 2],
        "wq": wq[core % 2], "wk": wk[core % 2], "wv": wv[core % 2],
        "wp": wp[core % 2], "ones": ones,
    } for core in range(8)]


def kernel_run(q, k, v, mask, Wq, Wk, Wv, Wp, bp, trace=False):
    """Run on hardware; returns (output, BassKernelResults)."""
    arrs = [np.asarray(x, dtype=np.float32)
            for x in (q, k, v, mask, Wq, Wk, Wv, Wp, bp)]
    q, k, v, mask, Wq, Wk, Wv, Wp, bp = arrs
    if "nc" not in _CACHE:
        _CACHE["nc"] = _build()
    nc = _CACHE["nc"]
    res = bass_utils.run_bass_kernel_spmd(
        nc, _in_maps(q, k, v, mask, Wq, Wk, Wv, Wp),
        core_ids=list(range(8)), trace=trace)
    out = np.empty((4, T, C), np.float32)
    for b in range(4):
        out[b] = res.results[2 * b]["y"] + res.results[2 * b + 1]["y"] + bp
    return out, res


def kernel(q, k, v, mask, Wq, Wk, Wv, Wp, bp):
    out, _ = kernel_run(q, k, v, mask, Wq, Wk, Wv, Wp, bp)
    return out
